# revision 28
# baseline (speedup 1.0000x reference)
"""Trainium2 Bass kernel for the BDH fast-weight recurrence (nn_BDH_GPU_36524401885328).

Mathematical reformulation (validated to ~9e-7 rel err vs the jax reference):
  - u_t = relu(token_emb[idx_t] @ Dx.T) >= 0, so the L1 normalizer of the x-scan
    is z_t = 0.97*||x_{t-1}||_1 + sum(u_t) + eps with ||x_{t-1}||_1 = 1 exactly
    (up to fp32 noise) => all z_t are computable in parallel from row sums.
  - the x recurrence x_t = (0.97*x_{t-1} + u_t)/z_t is an affine scan with known
    coefficients -> a single tensor_tensor_scan instruction per SBUF tile.
  - the fast-weight read a*_t = rho_{t-1} x_t unrolls to masked linear attention:
    a*_t = sum_{s<t} 0.97^(t-s) (x_s . x_t) vn_s  with vn_s = LN(token_emb[idx_s]).
  - A = (S o M) @ Vn is exactly zero-mean along d (Vn rows are LayerNormed), so
    LN(A) needs no mean subtraction; the final LN's mean subtraction is folded
    into a host-side pre-centered E (column means removed).
  Everything else (LayerNorms, Dy/E projections, relu gating) is pointwise in t.

Sharding: 8 cores = B(4) x T-halves(2). All cores run the SAME program; the
full-T prefix (UT, normalizers, x-scan, Vn) is computed everywhere, then each
core computes S/A/Yc/Out only for its 256-token half, selected by a
partition_id-driven dynamic-offset copy of X columns. Per-core decay masks
arrive as input data, so the program stays SPMD-uniform.
"""

import sys
import numpy as np

for _p in ("/opt/trn_rl_repo",):
    if _p not in sys.path:
        sys.path.insert(0, _p)

# If the surrounding process pinned jax to cpu (common in test harnesses),
# flip to the neuron/axon PJRT platform BEFORE concourse imports — the Bass
# runner needs the NeuronCore devices, and the backend can no longer be
# re-initialized after concourse's jax hooks load.
def _fix_jax_platform():
    try:
        import jax

        if jax.devices()[0].platform != "cpu":
            return
        for plat in ("axon", "neuron"):
            try:
                jax.config.update("jax_platforms", plat)
                from jax._src import xla_bridge

                xla_bridge._clear_backends()
                if jax.devices()[0].platform != "cpu":
                    return
            except Exception:
                continue
    except Exception:
        pass


_fix_jax_platform()

import concourse.bass as bass
import concourse.mybir as mybir
import concourse.tile as tile
from concourse import bacc, bass_utils

B, T, N, D, V = 4, 512, 1024, 256, 32000
U_DECAY = 0.97
X_DECAY = 0.97
EPS = 1e-6

F32 = mybir.dt.float32
BF16 = mybir.dt.bfloat16
ALU = mybir.AluOpType
ACTF = mybir.ActivationFunctionType

NT = N // 128   # 8 n tiles
DT = D // 128   # 2 d tiles
TT = T // 128   # 4 t tiles
TH = T // 2     # per-core token half


def _packed(d_ap, groups, width):
    """AP viewing a (groups*128, width) DRAM tensor as [128 part, groups, width]."""
    return bass.AP(
        tensor=d_ap.tensor,
        offset=0,
        ap=[[width, 128], [128 * width, groups], [1, width]],
    )


def build_nc():
    nc = bacc.Bacc("TRN2", target_bir_lowering=False, debug=False)

    # DRAM I/O (per core = one batch element)
    d_v = nc.dram_tensor("v_tm", [T, D], F32, kind="ExternalInput").ap()       # V token-major
    d_vth = nc.dram_tensor("v_fm_h", [D, T], BF16, kind="ExternalInput").ap()  # V^T bf16 hi
    d_vtl = nc.dram_tensor("v_fm_l", [D, T], BF16, kind="ExternalInput").ap()  # V^T bf16 lo
    d_dxth = nc.dram_tensor("dx_t_h", [D, N], BF16, kind="ExternalInput").ap()  # Dx^T bf16 hi
    d_dxtl = nc.dram_tensor("dx_t_l", [D, N], BF16, kind="ExternalInput").ap()  # Dx^T bf16 lo
    d_dyt = nc.dram_tensor("dy_t", [D, N], F32, kind="ExternalInput").ap()     # Dy^T
    d_et = nc.dram_tensor("e_t", [N, D], F32, kind="ExternalInput").ap()       # (E-colmean)^T
    d_mt = nc.dram_tensor("mask_ud", [T, TH], F32, kind="ExternalInput").ap()  # MT[s,tl]=UD^(toff+tl-s) masked
    d_ones = nc.dram_tensor("ones_col", [128, 1], F32, kind="ExternalInput").ap()
    d_onesb = nc.dram_tensor("ones_col_b", [128, 1], BF16, kind="ExternalInput").ap()
    d_onesr = nc.dram_tensor("ones_row", [1, 128], F32, kind="ExternalInput").ap()
    d_xdr = nc.dram_tensor("xdecay_row", [1, 128], F32, kind="ExternalInput").ap()
    d_out = nc.dram_tensor("out_ft", [D, TH], F32, kind="ExternalOutput").ap()  # Out^T (d, t half)

    with tile.TileContext(nc) as tc:
        with (
            tc.tile_pool(name="const", bufs=1) as const,
            tc.tile_pool(name="acts", bufs=1) as acts,
            tc.tile_pool(name="rows", bufs=1) as rows,
            tc.tile_pool(name="psbig", bufs=4, space="PSUM") as psbig,
            tc.tile_pool(name="psacc", bufs=2, space="PSUM") as psacc,
            tc.tile_pool(name="psrow", bufs=1, space="PSUM") as psrow,
            tc.tile_pool(name="psbc", bufs=1, space="PSUM") as psbc,
        ):
            # ---- inputs: one DMA per DRAM tensor, ordered by first use ------
            vth_all = const.tile([128, DT, T], BF16, tag="vth_all")
            vtl_all = const.tile([128, DT, T], BF16, tag="vtl_all")
            dxth_all = const.tile([128, DT, N], BF16, tag="dxth_all")
            dxtl_all = const.tile([128, DT, N], BF16, tag="dxtl_all")
            for k in range(DT):
                nc.sync.dma_start(out=vth_all[:, k, :], in_=_packed(d_vth, DT, T)[:, k, :])
                nc.sync.dma_start(out=dxth_all[:, k, :], in_=_packed(d_dxth, DT, N)[:, k, :])
                nc.sync.dma_start(out=vtl_all[:, k, :], in_=_packed(d_vtl, DT, T)[:, k, :])
                nc.sync.dma_start(out=dxtl_all[:, k, :], in_=_packed(d_dxtl, DT, N)[:, k, :])
            vth = [vth_all[:, k, :] for k in range(DT)]
            vtl = [vtl_all[:, k, :] for k in range(DT)]
            dxth = [dxth_all[:, k, :] for k in range(DT)]
            dxtl = [dxtl_all[:, k, :] for k in range(DT)]
            ones_col = const.tile([128, 1], F32, tag="ones")
            nc.sync.dma_start(out=ones_col, in_=d_ones)
            ones_col_b = const.tile([128, 1], BF16, tag="onesb")
            nc.sync.dma_start(out=ones_col_b, in_=d_onesb)
            ones_row = const.tile([1, 128], F32, tag="onesr")
            nc.sync.dma_start(out=ones_row, in_=d_onesr)
            xdecay_row = const.tile([1, 128], F32, tag="xdr")
            nc.sync.dma_start(out=xdecay_row, in_=d_xdr)
            v_all = const.tile([128, TT, D], F32, tag="v_all")
            nc.sync.dma_start(out=v_all, in_=_packed(d_v, TT, D))
            v_tm = [v_all[:, j, :] for j in range(TT)]
            mt_all = const.tile([128, TT, TH], F32, tag="mt_all")
            nc.sync.dma_start(out=mt_all, in_=_packed(d_mt, TT, TH))
            mt = [mt_all[:, j, :] for j in range(TT)]
            dyt_all = const.tile([128, DT, N], F32, tag="dyt_all")
            nc.sync.dma_start(out=dyt_all, in_=_packed(d_dyt, DT, N))
            dyt = [dyt_all[:, k, :] for k in range(DT)]
            et_all = const.tile([128, NT, D], F32, tag="et_all")
            nc.sync.dma_start(out=et_all, in_=_packed(d_et, NT, D))
            et = [et_all[:, i, :] for i in range(NT)]

            zero_c = const.tile([128, 1], F32, tag="zero_c")
            nc.vector.memset(zero_c, 0.0)
            eps_c = const.tile([128, 1], F32, tag="eps_c")
            nc.vector.memset(eps_c, EPS)
            zb_c = const.tile([128, 1], F32, tag="zb_c")
            nc.vector.memset(zb_c, X_DECAY + EPS)

            # ---- phase D: Vn = LN(V) rows (token-major) ---------------------
            vn = []
            for j in range(TT):
                stats = rows.tile([128, nc.vector.BN_STATS_DIM], F32, tag="bnst")
                nc.vector.bn_stats(out=stats, in_=v_tm[j])
                mv = rows.tile([128, nc.vector.BN_AGGR_DIM], F32, tag="bnag")
                nc.vector.bn_aggr(out=mv, in_=stats)
                std = rows.tile([128, 1], F32, tag="std")
                nc.scalar.activation(
                    std, mv[:, 1:2], ACTF.Sqrt, scale=float(D) / (D - 1),
                    bias=zero_c
                )
                stdeps = rows.tile([128, 1], F32, tag="stdeps")
                nc.scalar.activation(stdeps, std, ACTF.Identity, bias=eps_c)
                rstd = rows.tile([128, 1], F32, tag="rstd")
                rstd_scr = rows.tile([128, 1], F32, tag="rstd_scr")
                nc.vector.reciprocal_approx_accurate(rstd, stdeps, scratch=rstd_scr)
                vnj = acts.tile([128, D], F32, tag=f"vn{j}", name=f"vn{j}")
                nc.vector.tensor_scalar(
                    out=vnj,
                    in0=v_tm[j],
                    scalar1=mv[:, 0:1],
                    scalar2=rstd,
                    op0=ALU.subtract,
                    op1=ALU.mult,
                )
                vn.append(vnj)

            # ---- phase A: UT = relu(Dx @ V^T) / 0.97  (n-major) -------------
            uts = []
            uts_h = []
            uts_l = []
            for i in range(NT):
                ps = psbig.tile([128, T], F32, tag="big", name=f"ps_ut{i}")
                terms = [(dxth, vth), (dxth, vtl), (dxtl, vth)]
                nmm = len(terms) * DT
                w = 0
                for lhs, rhs in terms:
                    for k in range(DT):
                        nc.tensor.matmul(
                            ps,
                            lhs[k][:, i * 128:(i + 1) * 128],
                            rhs[k],
                            start=(w == 0),
                            stop=(w == nmm - 1),
                        )
                        w += 1
                u = acts.tile([128, T], F32, tag=f"uts{i}", name=f"uts{i}")
                nc.scalar.activation(u, ps, ACTF.Relu, scale=1.0 / X_DECAY, bias=zero_c)
                uts.append(u)
                uh = acts.tile([128, T], BF16, tag=f"uh{i}", name=f"uh{i}")
                nc.scalar.copy(uh, u)
                ul = acts.tile([128, T], BF16, tag=f"ul{i}", name=f"ul{i}")
                nc.vector.tensor_sub(ul, u, uh)
                uts_h.append(uh)
                uts_l.append(ul)

            # ---- phase B: normalizer row z_t --------------------------------
            ps_s = psrow.tile([1, T], F32, tag="row")
            for i in range(NT):
                nc.tensor.matmul(
                    ps_s, ones_col_b, uts_h[i], start=(i == 0), stop=False
                )
                nc.tensor.matmul(
                    ps_s, ones_col_b, uts_l[i], start=False, stop=(i == NT - 1)
                )
            zrow = rows.tile([1, T], F32, tag="zrow")
            # z = 0.97*s' + (0.97 + eps)   [s' = s/0.97 from the folded relu scale]
            nc.scalar.activation(
                zrow, ps_s, ACTF.Identity, scale=X_DECAY, bias=zb_c[:1]
            )
            # t=0: x_{-1}=0 -> z_0 = s_0 + eps
            nc.scalar.activation(
                zrow[:, 0:1], ps_s[:, 0:1], ACTF.Identity, scale=X_DECAY,
                bias=eps_c[:1]
            )
            rz = rows.tile([1, T], F32, tag="rz")
            nc.vector.reciprocal_approx_fast(rz, zrow)
            abc = psbc.tile([128, T], F32, tag="bc", name="abc")
            nc.tensor.matmul(abc, xdecay_row, rz, start=True, stop=True)

            # ---- phase C: x scan (exact affine recurrence along t) ----------
            xt = []
            for i in range(NT):
                x = acts.tile([128, T], F32, tag=f"xt{i}", name=f"xt{i}")
                nc.vector.tensor_tensor_scan(
                    x, uts[i], abc, initial=0.0, op0=ALU.add, op1=ALU.mult
                )
                xt.append(x)

            # ---- my T-half of X, selected by partition id -------------------
            pid = nc.partition_id()
            toff = (pid % 2) * TH
            xh = []
            for i in range(NT):
                h = acts.tile([128, TH], F32, tag=f"xh{i}", name=f"xh{i}")
                nc.sync.dma_start(out=h, in_=xt[i][:, bass.ds(toff, TH)])
                xh.append(h)

            # ---- phase E: S[t', t] = sum_n X[t',n] X[t,n] (t >= t' band) ----
            # k-major loop order: S accumulation pipelines behind the scans.
            ps_sj = [
                psbig.tile([128, TH], F32, tag="big", name=f"ps_s{j}")
                for j in range(TT)
            ]
            for k in range(NT):
                for j in range(TT):
                    nc.tensor.matmul(
                        ps_sj[j],
                        xt[k][:, j * 128:(j + 1) * 128],
                        xh[k],
                        start=(k == 0),
                        stop=(k == NT - 1),
                    )
            smt = []
            for j in range(TT):
                sm = acts.tile([128, TH], F32, tag=f"smt{j}", name=f"smt{j}")
                nc.vector.tensor_mul(sm, ps_sj[j], mt[j])
                smt.append(sm)

            # ---- phase F: A^T = Vn^T (S o M)^T; LN along d needs no mean ----
            ps_at = [psacc.tile([128, TH], F32, tag="acc", name=f"ps_at{m}") for m in range(DT)]
            for m in range(DT):
                for j in range(TT):
                    nc.tensor.matmul(
                        ps_at[m],
                        vn[j][:, m * 128:(m + 1) * 128],
                        smt[j],
                        start=(j == 0),
                        stop=(j == TT - 1),
                    )
            # centered A straight to SBUF; the 1/(std+eps) scale is deferred
            # through the (linear) Yc/relu-gate/Out chain to the final scaling.
            atc = []
            for m in range(DT):
                a = acts.tile([128, TH], F32, tag=f"atn{m}", name=f"atn{m}")
                nc.scalar.copy(a, ps_at[m])
                atc.append(a)
            sq_a = []
            for m in range(DT):
                sq = acts.tile([128, TH], F32, tag=f"sq_a{m}", name=f"sq_a{m}")
                nc.scalar.activation(sq, ps_at[m], ACTF.Square, bias=zero_c)
                sq_a.append(sq)
            ps_ssa = psrow.tile([1, TH], F32, tag="row")
            for m in range(DT):
                nc.tensor.matmul(
                    ps_ssa, ones_col, sq_a[m], start=(m == 0), stop=(m == DT - 1)
                )
            srow_a = rows.tile([1, TH], F32, tag="srow_a")
            nc.scalar.activation(
                srow_a, ps_ssa, ACTF.Sqrt, scale=1.0 / (D - 1), bias=zero_c[:1]
            )
            seps_a = rows.tile([1, TH], F32, tag="seps_a")
            nc.scalar.activation(seps_a, srow_a, ACTF.Identity, bias=eps_c[:1])
            rinv_a = rows.tile([1, TH], F32, tag="rinv_a")
            nc.vector.reciprocal_approx_fast(rinv_a, seps_a)

            # ---- phase G: Yc = Dy @ LN(A); Ytl = relu(Yc) * X ---------------
            ytl = []
            for i in range(NT):
                ps = psbig.tile([128, TH], F32, tag="big", name=f"ps_yc{i}")
                for m in range(DT):
                    nc.tensor.matmul(
                        ps,
                        dyt[m][:, i * 128:(i + 1) * 128],
                        atc[m],
                        start=(m == 0),
                        stop=(m == DT - 1),
                    )
                y = acts.tile([128, TH], F32, tag=f"ytl{i}", name=f"ytl{i}")
                nc.vector.scalar_tensor_tensor(
                    out=y, in0=ps, scalar=0.0, in1=xh[i], op0=ALU.max, op1=ALU.mult
                )
                ytl.append(y)

            # ---- phase H: Out^T = Ec @ Ytl (pre-centered), final LN ---------
            ps_o = [psacc.tile([128, TH], F32, tag="acc", name=f"ps_o{m}") for m in range(DT)]
            for m in range(DT):
                for i in range(NT):
                    nc.tensor.matmul(
                        ps_o[m],
                        et[i][:, m * 128:(m + 1) * 128],
                        ytl[i],
                        start=(i == 0),
                        stop=(i == NT - 1),
                    )
            sq_o = []
            for m in range(DT):
                sq = acts.tile([128, TH], F32, tag=f"sq_o{m}", name=f"sq_o{m}")
                nc.scalar.activation(sq, ps_o[m], ACTF.Square, bias=zero_c)
                sq_o.append(sq)
            ps_sso = psrow.tile([1, TH], F32, tag="row")
            for m in range(DT):
                nc.tensor.matmul(
                    ps_sso, ones_col, sq_o[m], start=(m == 0), stop=(m == DT - 1)
                )
            # ps_sso holds ssq of the UN-rescaled OutC_raw = rinv_a^-1 * Out_true.
            # true scale = rinv_a / (rinv_a*sqrt(ssq_raw/255) + eps).
            srow_o = rows.tile([1, TH], F32, tag="srow_o")
            nc.scalar.activation(
                srow_o, ps_sso, ACTF.Sqrt, scale=1.0 / (D - 1), bias=zero_c[:1]
            )
            seps_o = rows.tile([1, TH], F32, tag="seps_o")
            nc.vector.scalar_tensor_tensor(
                out=seps_o, in0=srow_o, scalar=0.0, in1=rinv_a,
                op0=ALU.add, op1=ALU.mult,
            )
            nc.scalar.activation(seps_o, seps_o, ACTF.Identity, bias=eps_c[:1])
            rinv_o = rows.tile([1, TH], F32, tag="rinv_o")
            rio_scr = rows.tile([1, TH], F32, tag="rio_scr")
            nc.vector.reciprocal_approx_accurate(rinv_o, seps_o, scratch=rio_scr)
            scale_row = rows.tile([1, TH], F32, tag="scale_row")
            nc.vector.tensor_mul(scale_row, rinv_o, rinv_a)
            rb_o_ps = psbc.tile([128, TH], F32, tag="bc", name="rb_o_ps")
            nc.tensor.matmul(rb_o_ps, ones_row, scale_row, start=True, stop=True)
            rb_o = acts.tile([128, TH], F32, tag="rb_o")
            nc.scalar.copy(rb_o, rb_o_ps)
            for m in range(DT):
                o = acts.tile([128, TH], F32, tag=f"outt{m}", name=f"outt{m}")
                nc.vector.tensor_mul(o, ps_o[m], rb_o)
                nc.sync.dma_start(out=d_out[m * 128:(m + 1) * 128, :], in_=o)

    nc.compile()
    return nc


def host_inputs(idx, token_emb, E, Dx, Dy):
    """Build the per-core input maps (one batch element per core)."""
    idx = np.asarray(idx)
    token_emb = np.ascontiguousarray(np.asarray(token_emb, dtype=np.float32))
    E = np.asarray(E, dtype=np.float32)
    Dx = np.asarray(Dx, dtype=np.float32)
    Dy = np.asarray(Dy, dtype=np.float32)

    import ml_dtypes

    def _hilo(x):
        h = x.astype(ml_dtypes.bfloat16)
        l = (x - h.astype(np.float32)).astype(ml_dtypes.bfloat16)
        return h, l

    dx_t_h, dx_t_l = _hilo(np.ascontiguousarray(Dx.T))
    dy_t = np.ascontiguousarray(Dy.T)
    Ec = (E - E.mean(axis=0, keepdims=True, dtype=np.float32)).astype(np.float32)
    e_t = np.ascontiguousarray(Ec.T)
    tt = np.arange(T)
    diff = tt[None, :] - tt[:, None]            # [s, t] = t - s
    mask = np.where(diff > 0, U_DECAY ** np.maximum(diff, 0), 0.0).astype(np.float32)
    masks = [np.ascontiguousarray(mask[:, h * TH:(h + 1) * TH]) for h in range(2)]
    ones_col = np.ones((128, 1), np.float32)
    ones_col_b = np.ones((128, 1), ml_dtypes.bfloat16)
    ones_row = np.ones((1, 128), np.float32)
    xdecay_row = np.full((1, 128), X_DECAY, np.float32)

    in_maps = []
    for b in range(B):
        v = np.ascontiguousarray(token_emb[idx[b]])         # (T, D)
        vt_h, vt_l = _hilo(np.ascontiguousarray(v.T))
        for h in range(2):
            in_maps.append(
                {
                    "v_tm": v,
                    "v_fm_h": vt_h,
                    "v_fm_l": vt_l,
                    "dx_t_h": dx_t_h,
                    "dx_t_l": dx_t_l,
                    "dy_t": dy_t,
                    "e_t": e_t,
                    "mask_ud": masks[h],
                    "ones_col": ones_col,
                    "ones_col_b": ones_col_b,
                    "ones_row": ones_row,
                    "xdecay_row": xdecay_row,
                }
            )
    return in_maps


_NC_CACHE = None


def _neuron_devices_ok():
    try:
        import jax

        return len(jax.devices()) >= 2 * B and jax.devices()[0].platform != "cpu"
    except Exception:
        return False


def _kernel_via_subprocess(idx, token_emb, E, Dx, Dy):
    """Fallback when the calling process's jax is pinned to cpu: run in a
    clean interpreter where the default (axon/neuron) platform initializes."""
    import os
    import pickle
    import subprocess
    import sys as _sys
    import tempfile

    d = tempfile.mkdtemp(prefix="bdh_kernel_")
    inp = os.path.join(d, "in.npz")
    outp = os.path.join(d, "out.npy")
    np.savez(inp, idx=idx, token_emb=token_emb, E=E, Dx=Dx, Dy=Dy)
    code = (
        "import numpy as np, sys\n"
        f"sys.path.insert(0, {os.path.dirname(os.path.abspath(__file__))!r})\n"
        "import kernel as K\n"
        f"z = np.load({inp!r})\n"
        "out = K.kernel(idx=z['idx'], token_emb=z['token_emb'], E=z['E'], "
        "Dx=z['Dx'], Dy=z['Dy'])\n"
        f"np.save({outp!r}, out)\n"
    )
    env = dict(os.environ)
    env.pop("JAX_PLATFORMS", None)
    subprocess.run([_sys.executable, "-c", code], env=env, check=True)
    return np.load(outp)


def kernel(idx, token_emb, E, Dx, Dy, _return_results=False, **run_kwargs):
    global _NC_CACHE
    _fix_jax_platform()
    if not _neuron_devices_ok():
        out = _kernel_via_subprocess(idx, token_emb, E, Dx, Dy)
        if _return_results:
            return out, None
        return out
    if _NC_CACHE is None:
        _NC_CACHE = build_nc()
    nc = _NC_CACHE
    in_maps = host_inputs(idx, token_emb, E, Dx, Dy)
    res = bass_utils.run_bass_kernel_spmd(
        nc, in_maps, core_ids=list(range(2 * B)), **run_kwargs
    )
    out = np.zeros((B, T, D), dtype=np.float32)
    for b in range(B):
        for h in range(2):
            out[b, h * TH:(h + 1) * TH] = res.results[2 * b + h]["out_ft"].T
    if _return_results:
        return out, res
    return out


# revision 29
# speedup vs baseline: 1.2099x; 1.2099x over previous
"""Trainium2 Bass kernel for the BDH fast-weight recurrence (nn_BDH_GPU_36524401885328).

Mathematical reformulation (validated to ~9e-7 rel err vs the jax reference):
  - u_t = relu(token_emb[idx_t] @ Dx.T) >= 0, so the L1 normalizer of the x-scan
    is z_t = 0.97*||x_{t-1}||_1 + sum(u_t) + eps with ||x_{t-1}||_1 = 1 exactly
    (up to fp32 noise) => all z_t are computable in parallel from row sums.
  - the x recurrence x_t = (0.97*x_{t-1} + u_t)/z_t is an affine scan with known
    coefficients -> a single tensor_tensor_scan instruction per SBUF tile.
  - the fast-weight read a*_t = rho_{t-1} x_t unrolls to masked linear attention:
    a*_t = sum_{s<t} 0.97^(t-s) (x_s . x_t) vn_s  with vn_s = LN(token_emb[idx_s]).
  - A = (S o M) @ Vn is exactly zero-mean along d (Vn rows are LayerNormed), so
    LN(A) needs no mean subtraction; the final LN's mean subtraction is folded
    into a host-side pre-centered E (column means removed).
  Everything else (LayerNorms, Dy/E projections, relu gating) is pointwise in t.

Sharding: 8 cores = B(4) x T-halves(2). All cores run the SAME program; the
full-T prefix (UT, normalizers, x-scan, Vn) is computed everywhere, then each
core computes S/A/Yc/Out only for its 256-token half, selected by a
partition_id-driven dynamic-offset copy of X columns. Per-core decay masks
arrive as input data, so the program stays SPMD-uniform.
"""

import sys
import numpy as np

for _p in ("/opt/trn_rl_repo",):
    if _p not in sys.path:
        sys.path.insert(0, _p)

# If the surrounding process pinned jax to cpu (common in test harnesses),
# flip to the neuron/axon PJRT platform BEFORE concourse imports — the Bass
# runner needs the NeuronCore devices, and the backend can no longer be
# re-initialized after concourse's jax hooks load.
def _fix_jax_platform():
    try:
        import jax

        if jax.devices()[0].platform != "cpu":
            return
        for plat in ("axon", "neuron"):
            try:
                jax.config.update("jax_platforms", plat)
                from jax._src import xla_bridge

                xla_bridge._clear_backends()
                if jax.devices()[0].platform != "cpu":
                    return
            except Exception:
                continue
    except Exception:
        pass


_fix_jax_platform()

import concourse.bass as bass
import concourse.mybir as mybir
import concourse.tile as tile
from concourse import bacc, bass_utils

B, T, N, D, V = 4, 512, 1024, 256, 32000
U_DECAY = 0.97
X_DECAY = 0.97
EPS = 1e-6

F32 = mybir.dt.float32
BF16 = mybir.dt.bfloat16
ALU = mybir.AluOpType
ACTF = mybir.ActivationFunctionType

NT = N // 128   # 8 n tiles
DT = D // 128   # 2 d tiles
TT = T // 128   # 4 t tiles
TH = T // 2     # per-core token half


def _packed(d_ap, groups, width):
    """AP viewing a (groups*128, width) DRAM tensor as [128 part, groups, width]."""
    return bass.AP(
        tensor=d_ap.tensor,
        offset=0,
        ap=[[width, 128], [128 * width, groups], [1, width]],
    )


def build_nc():
    nc = bacc.Bacc("TRN2", target_bir_lowering=False, debug=False)

    # DRAM I/O (per core = one batch element)
    d_v = nc.dram_tensor("v_tm", [T, D], F32, kind="ExternalInput").ap()       # V token-major
    d_vth = nc.dram_tensor("v_fm_h", [D, T], BF16, kind="ExternalInput").ap()  # V^T bf16 hi
    d_vtl = nc.dram_tensor("v_fm_l", [D, T], BF16, kind="ExternalInput").ap()  # V^T bf16 lo
    d_dxth = nc.dram_tensor("dx_t_h", [D, N], BF16, kind="ExternalInput").ap()  # Dx^T bf16 hi
    d_dxtl = nc.dram_tensor("dx_t_l", [D, N], BF16, kind="ExternalInput").ap()  # Dx^T bf16 lo
    d_dyt = nc.dram_tensor("dy_t", [D, N], F32, kind="ExternalInput").ap()     # Dy^T
    d_et = nc.dram_tensor("e_t", [N, D], F32, kind="ExternalInput").ap()       # (E-colmean)^T
    d_mt = nc.dram_tensor("mask_ud", [T, TH], F32, kind="ExternalInput").ap()  # MT[s,tl]=UD^(toff+tl-s) masked
    d_ones = nc.dram_tensor("ones_col", [128, 1], F32, kind="ExternalInput").ap()
    d_onesb = nc.dram_tensor("ones_col_b", [128, 1], BF16, kind="ExternalInput").ap()
    d_onesr = nc.dram_tensor("ones_row", [1, 128], F32, kind="ExternalInput").ap()
    d_xdr = nc.dram_tensor("xdecay_row", [1, 128], F32, kind="ExternalInput").ap()
    d_out = nc.dram_tensor("out_ft", [D, TH], F32, kind="ExternalOutput").ap()  # Out^T (d, t half)

    with tile.TileContext(nc) as tc:
        with (
            tc.tile_pool(name="const", bufs=1) as const,
            tc.tile_pool(name="acts", bufs=1) as acts,
            tc.tile_pool(name="rows", bufs=1) as rows,
            tc.tile_pool(name="psbig", bufs=4, space="PSUM") as psbig,
            tc.tile_pool(name="psacc", bufs=2, space="PSUM") as psacc,
            tc.tile_pool(name="psrow", bufs=1, space="PSUM") as psrow,
            tc.tile_pool(name="psbc", bufs=1, space="PSUM") as psbc,
        ):
            # ---- inputs: one DMA per DRAM tensor, ordered by first use ------
            vth_all = const.tile([128, DT, T], BF16, tag="vth_all")
            vtl_all = const.tile([128, DT, T], BF16, tag="vtl_all")
            dxth_all = const.tile([128, DT, N], BF16, tag="dxth_all")
            dxtl_all = const.tile([128, DT, N], BF16, tag="dxtl_all")
            for k in range(DT):
                nc.sync.dma_start(out=vth_all[:, k, :], in_=_packed(d_vth, DT, T)[:, k, :])
                nc.sync.dma_start(out=dxth_all[:, k, :], in_=_packed(d_dxth, DT, N)[:, k, :])
                nc.sync.dma_start(out=vtl_all[:, k, :], in_=_packed(d_vtl, DT, T)[:, k, :])
                nc.sync.dma_start(out=dxtl_all[:, k, :], in_=_packed(d_dxtl, DT, N)[:, k, :])
            vth = [vth_all[:, k, :] for k in range(DT)]
            vtl = [vtl_all[:, k, :] for k in range(DT)]
            dxth = [dxth_all[:, k, :] for k in range(DT)]
            dxtl = [dxtl_all[:, k, :] for k in range(DT)]
            ones_col = const.tile([128, 1], F32, tag="ones")
            nc.sync.dma_start(out=ones_col, in_=d_ones)
            ones_col_b = const.tile([128, 1], BF16, tag="onesb")
            nc.sync.dma_start(out=ones_col_b, in_=d_onesb)
            ones_row = const.tile([1, 128], F32, tag="onesr")
            nc.sync.dma_start(out=ones_row, in_=d_onesr)
            xdecay_row = const.tile([1, 128], F32, tag="xdr")
            nc.sync.dma_start(out=xdecay_row, in_=d_xdr)
            v_all = const.tile([128, TT, D], F32, tag="v_all")
            nc.sync.dma_start(out=v_all, in_=_packed(d_v, TT, D))
            v_tm = [v_all[:, j, :] for j in range(TT)]
            mt_all = const.tile([128, TT, TH], F32, tag="mt_all")
            nc.sync.dma_start(out=mt_all, in_=_packed(d_mt, TT, TH))
            mt = [mt_all[:, j, :] for j in range(TT)]
            dyt_all = const.tile([128, DT, N], F32, tag="dyt_all")
            nc.sync.dma_start(out=dyt_all, in_=_packed(d_dyt, DT, N))
            dyt = [dyt_all[:, k, :] for k in range(DT)]
            et_all = const.tile([128, NT, D], F32, tag="et_all")
            nc.sync.dma_start(out=et_all, in_=_packed(d_et, NT, D))
            et = [et_all[:, i, :] for i in range(NT)]

            zero_c = const.tile([128, 1], F32, tag="zero_c")
            nc.vector.memset(zero_c, 0.0)
            eps_c = const.tile([128, 1], F32, tag="eps_c")
            nc.vector.memset(eps_c, EPS)
            zb_c = const.tile([128, 1], F32, tag="zb_c")
            nc.vector.memset(zb_c, X_DECAY + EPS)

            # ---- phase A: UT = relu(Dx @ V^T) / 0.97  (n-major) -------------
            uts = []
            uts_h = []
            uts_l = []
            for i in range(NT):
                ps = psbig.tile([128, T], F32, tag="big", name=f"ps_ut{i}")
                terms = [(dxth, vth), (dxth, vtl), (dxtl, vth)]
                nmm = len(terms) * DT
                w = 0
                for lhs, rhs in terms:
                    for k in range(DT):
                        nc.tensor.matmul(
                            ps,
                            lhs[k][:, i * 128:(i + 1) * 128],
                            rhs[k],
                            start=(w == 0),
                            stop=(w == nmm - 1),
                        )
                        w += 1
                u = acts.tile([128, T], F32, tag=f"uts{i}", name=f"uts{i}")
                nc.scalar.activation(u, ps, ACTF.Relu, scale=1.0 / X_DECAY, bias=zero_c)
                uts.append(u)
                uh = acts.tile([128, T], BF16, tag=f"uh{i}", name=f"uh{i}")
                nc.scalar.copy(uh, u)
                ul = acts.tile([128, T], BF16, tag=f"ul{i}", name=f"ul{i}")
                nc.vector.tensor_sub(ul, u, uh)
                uts_h.append(uh)
                uts_l.append(ul)

            # ---- phase B: normalizer row z_t --------------------------------
            ps_s = psrow.tile([1, T], F32, tag="row")
            for i in range(NT):
                nc.tensor.matmul(
                    ps_s, ones_col_b, uts_h[i], start=(i == 0), stop=False
                )
                nc.tensor.matmul(
                    ps_s, ones_col_b, uts_l[i], start=False, stop=(i == NT - 1)
                )
            zrow = rows.tile([1, T], F32, tag="zrow")
            # z = 0.97*s' + (0.97 + eps)   [s' = s/0.97 from the folded relu scale]
            nc.scalar.activation(
                zrow, ps_s, ACTF.Identity, scale=X_DECAY, bias=zb_c[:1]
            )
            # t=0: x_{-1}=0 -> z_0 = s_0 + eps
            nc.scalar.activation(
                zrow[:, 0:1], ps_s[:, 0:1], ACTF.Identity, scale=X_DECAY,
                bias=eps_c[:1]
            )
            rz = rows.tile([1, T], F32, tag="rz")
            nc.vector.reciprocal_approx_fast(rz, zrow)
            abc = psbc.tile([128, T], F32, tag="bc", name="abc")
            nc.tensor.matmul(abc, xdecay_row, rz, start=True, stop=True)

            # ---- phase C: x scan (exact affine recurrence along t) ----------
            xt = []
            for i in range(NT):
                x = acts.tile([128, T], F32, tag=f"xt{i}", name=f"xt{i}")
                nc.vector.tensor_tensor_scan(
                    x, uts[i], abc, initial=0.0, op0=ALU.add, op1=ALU.mult
                )
                xt.append(x)

            # ---- my T-half of X, selected by partition id -------------------
            pid = nc.partition_id()
            toff = (pid % 2) * TH
            xh = []
            for i in range(NT):
                h = acts.tile([128, TH], F32, tag=f"xh{i}", name=f"xh{i}")
                nc.sync.dma_start(out=h, in_=xt[i][:, bass.ds(toff, TH)])
                xh.append(h)

            # ---- phase D: Vn = LN(V) rows (token-major) ---------------------
            vn = []
            for j in range(TT):
                stats = rows.tile([128, nc.vector.BN_STATS_DIM], F32, tag="bnst")
                nc.vector.bn_stats(out=stats, in_=v_tm[j])
                mv = rows.tile([128, nc.vector.BN_AGGR_DIM], F32, tag="bnag")
                nc.vector.bn_aggr(out=mv, in_=stats)
                std = rows.tile([128, 1], F32, tag="std")
                nc.scalar.activation(
                    std, mv[:, 1:2], ACTF.Sqrt, scale=float(D) / (D - 1),
                    bias=zero_c
                )
                stdeps = rows.tile([128, 1], F32, tag="stdeps")
                nc.scalar.activation(stdeps, std, ACTF.Identity, bias=eps_c)
                rstd = rows.tile([128, 1], F32, tag="rstd")
                rstd_scr = rows.tile([128, 1], F32, tag="rstd_scr")
                nc.vector.reciprocal_approx_accurate(rstd, stdeps, scratch=rstd_scr)
                vnj = acts.tile([128, D], F32, tag=f"vn{j}", name=f"vn{j}")
                nc.vector.tensor_scalar(
                    out=vnj,
                    in0=v_tm[j],
                    scalar1=mv[:, 0:1],
                    scalar2=rstd,
                    op0=ALU.subtract,
                    op1=ALU.mult,
                )
                vn.append(vnj)

            # ---- phase E: S[t', t] = sum_n X[t',n] X[t,n] (t >= t' band) ----
            # k-major loop order: S accumulation pipelines behind the scans.
            ps_sj = [
                psbig.tile([128, TH], F32, tag="big", name=f"ps_s{j}")
                for j in range(TT)
            ]
            for k in range(NT):
                for j in range(TT):
                    nc.tensor.matmul(
                        ps_sj[j],
                        xt[k][:, j * 128:(j + 1) * 128],
                        xh[k],
                        start=(k == 0),
                        stop=(k == NT - 1),
                    )
            smt = []
            for j in range(TT):
                sm = acts.tile([128, TH], F32, tag=f"smt{j}", name=f"smt{j}")
                nc.vector.tensor_mul(sm, ps_sj[j], mt[j])
                smt.append(sm)

            # ---- phase F: A^T = Vn^T (S o M)^T; LN along d needs no mean ----
            ps_at = [psacc.tile([128, TH], F32, tag="acc", name=f"ps_at{m}") for m in range(DT)]
            for m in range(DT):
                for j in range(TT):
                    nc.tensor.matmul(
                        ps_at[m],
                        vn[j][:, m * 128:(m + 1) * 128],
                        smt[j],
                        start=(j == 0),
                        stop=(j == TT - 1),
                    )
            # centered A straight to SBUF; the 1/(std+eps) scale is deferred
            # through the (linear) Yc/relu-gate/Out chain to the final scaling.
            atc = []
            for m in range(DT):
                a = acts.tile([128, TH], F32, tag=f"atn{m}", name=f"atn{m}")
                nc.scalar.copy(a, ps_at[m])
                atc.append(a)
            sq_a = []
            for m in range(DT):
                sq = acts.tile([128, TH], F32, tag=f"sq_a{m}", name=f"sq_a{m}")
                nc.scalar.activation(sq, ps_at[m], ACTF.Square, bias=zero_c)
                sq_a.append(sq)
            ps_ssa = psrow.tile([1, TH], F32, tag="row")
            for m in range(DT):
                nc.tensor.matmul(
                    ps_ssa, ones_col, sq_a[m], start=(m == 0), stop=(m == DT - 1)
                )
            srow_a = rows.tile([1, TH], F32, tag="srow_a")
            nc.scalar.activation(
                srow_a, ps_ssa, ACTF.Sqrt, scale=1.0 / (D - 1), bias=zero_c[:1]
            )
            seps_a = rows.tile([1, TH], F32, tag="seps_a")
            nc.scalar.activation(seps_a, srow_a, ACTF.Identity, bias=eps_c[:1])
            rinv_a = rows.tile([1, TH], F32, tag="rinv_a")
            nc.vector.reciprocal_approx_fast(rinv_a, seps_a)

            # ---- phase G: Yc = Dy @ LN(A); Ytl = relu(Yc) * X ---------------
            ytl = []
            for i in range(NT):
                ps = psbig.tile([128, TH], F32, tag="big", name=f"ps_yc{i}")
                for m in range(DT):
                    nc.tensor.matmul(
                        ps,
                        dyt[m][:, i * 128:(i + 1) * 128],
                        atc[m],
                        start=(m == 0),
                        stop=(m == DT - 1),
                    )
                y = acts.tile([128, TH], F32, tag=f"ytl{i}", name=f"ytl{i}")
                nc.vector.scalar_tensor_tensor(
                    out=y, in0=ps, scalar=0.0, in1=xh[i], op0=ALU.max, op1=ALU.mult
                )
                ytl.append(y)

            # ---- phase H: Out^T = Ec @ Ytl (pre-centered), final LN ---------
            ps_o = [psacc.tile([128, TH], F32, tag="acc", name=f"ps_o{m}") for m in range(DT)]
            for m in range(DT):
                for i in range(NT):
                    nc.tensor.matmul(
                        ps_o[m],
                        et[i][:, m * 128:(m + 1) * 128],
                        ytl[i],
                        start=(i == 0),
                        stop=(i == NT - 1),
                    )
            sq_o = []
            for m in range(DT):
                sq = acts.tile([128, TH], F32, tag=f"sq_o{m}", name=f"sq_o{m}")
                nc.scalar.activation(sq, ps_o[m], ACTF.Square, bias=zero_c)
                sq_o.append(sq)
            ps_sso = psrow.tile([1, TH], F32, tag="row")
            for m in range(DT):
                nc.tensor.matmul(
                    ps_sso, ones_col, sq_o[m], start=(m == 0), stop=(m == DT - 1)
                )
            # ps_sso holds ssq of the UN-rescaled OutC_raw = rinv_a^-1 * Out_true.
            # true scale = rinv_a / (rinv_a*sqrt(ssq_raw/255) + eps).
            srow_o = rows.tile([1, TH], F32, tag="srow_o")
            nc.scalar.activation(
                srow_o, ps_sso, ACTF.Sqrt, scale=1.0 / (D - 1), bias=zero_c[:1]
            )
            seps_o = rows.tile([1, TH], F32, tag="seps_o")
            nc.vector.scalar_tensor_tensor(
                out=seps_o, in0=srow_o, scalar=0.0, in1=rinv_a,
                op0=ALU.add, op1=ALU.mult,
            )
            nc.scalar.activation(seps_o, seps_o, ACTF.Identity, bias=eps_c[:1])
            rinv_o = rows.tile([1, TH], F32, tag="rinv_o")
            rio_scr = rows.tile([1, TH], F32, tag="rio_scr")
            nc.vector.reciprocal_approx_accurate(rinv_o, seps_o, scratch=rio_scr)
            scale_row = rows.tile([1, TH], F32, tag="scale_row")
            nc.vector.tensor_mul(scale_row, rinv_o, rinv_a)
            rb_o_ps = psbc.tile([128, TH], F32, tag="bc", name="rb_o_ps")
            nc.tensor.matmul(rb_o_ps, ones_row, scale_row, start=True, stop=True)
            rb_o = acts.tile([128, TH], F32, tag="rb_o")
            nc.scalar.copy(rb_o, rb_o_ps)
            for m in range(DT):
                o = acts.tile([128, TH], F32, tag=f"outt{m}", name=f"outt{m}")
                nc.vector.tensor_mul(o, ps_o[m], rb_o)
                nc.sync.dma_start(out=d_out[m * 128:(m + 1) * 128, :], in_=o)

    nc.compile()
    return nc


def host_inputs(idx, token_emb, E, Dx, Dy):
    """Build the per-core input maps (one batch element per core)."""
    idx = np.asarray(idx)
    token_emb = np.ascontiguousarray(np.asarray(token_emb, dtype=np.float32))
    E = np.asarray(E, dtype=np.float32)
    Dx = np.asarray(Dx, dtype=np.float32)
    Dy = np.asarray(Dy, dtype=np.float32)

    import ml_dtypes

    def _hilo(x):
        h = x.astype(ml_dtypes.bfloat16)
        l = (x - h.astype(np.float32)).astype(ml_dtypes.bfloat16)
        return h, l

    dx_t_h, dx_t_l = _hilo(np.ascontiguousarray(Dx.T))
    dy_t = np.ascontiguousarray(Dy.T)
    Ec = (E - E.mean(axis=0, keepdims=True, dtype=np.float32)).astype(np.float32)
    e_t = np.ascontiguousarray(Ec.T)
    tt = np.arange(T)
    diff = tt[None, :] - tt[:, None]            # [s, t] = t - s
    mask = np.where(diff > 0, U_DECAY ** np.maximum(diff, 0), 0.0).astype(np.float32)
    masks = [np.ascontiguousarray(mask[:, h * TH:(h + 1) * TH]) for h in range(2)]
    ones_col = np.ones((128, 1), np.float32)
    ones_col_b = np.ones((128, 1), ml_dtypes.bfloat16)
    ones_row = np.ones((1, 128), np.float32)
    xdecay_row = np.full((1, 128), X_DECAY, np.float32)

    in_maps = []
    for b in range(B):
        v = np.ascontiguousarray(token_emb[idx[b]])         # (T, D)
        vt_h, vt_l = _hilo(np.ascontiguousarray(v.T))
        for h in range(2):
            in_maps.append(
                {
                    "v_tm": v,
                    "v_fm_h": vt_h,
                    "v_fm_l": vt_l,
                    "dx_t_h": dx_t_h,
                    "dx_t_l": dx_t_l,
                    "dy_t": dy_t,
                    "e_t": e_t,
                    "mask_ud": masks[h],
                    "ones_col": ones_col,
                    "ones_col_b": ones_col_b,
                    "ones_row": ones_row,
                    "xdecay_row": xdecay_row,
                }
            )
    return in_maps


_NC_CACHE = None


def _neuron_devices_ok():
    try:
        import jax

        return len(jax.devices()) >= 2 * B and jax.devices()[0].platform != "cpu"
    except Exception:
        return False


def _kernel_via_subprocess(idx, token_emb, E, Dx, Dy):
    """Fallback when the calling process's jax is pinned to cpu: run in a
    clean interpreter where the default (axon/neuron) platform initializes."""
    import os
    import pickle
    import subprocess
    import sys as _sys
    import tempfile

    d = tempfile.mkdtemp(prefix="bdh_kernel_")
    inp = os.path.join(d, "in.npz")
    outp = os.path.join(d, "out.npy")
    np.savez(inp, idx=idx, token_emb=token_emb, E=E, Dx=Dx, Dy=Dy)
    code = (
        "import numpy as np, sys\n"
        f"sys.path.insert(0, {os.path.dirname(os.path.abspath(__file__))!r})\n"
        "import kernel as K\n"
        f"z = np.load({inp!r})\n"
        "out = K.kernel(idx=z['idx'], token_emb=z['token_emb'], E=z['E'], "
        "Dx=z['Dx'], Dy=z['Dy'])\n"
        f"np.save({outp!r}, out)\n"
    )
    env = dict(os.environ)
    env.pop("JAX_PLATFORMS", None)
    subprocess.run([_sys.executable, "-c", code], env=env, check=True)
    return np.load(outp)


def kernel(idx, token_emb, E, Dx, Dy, _return_results=False, **run_kwargs):
    global _NC_CACHE
    _fix_jax_platform()
    if not _neuron_devices_ok():
        out = _kernel_via_subprocess(idx, token_emb, E, Dx, Dy)
        if _return_results:
            return out, None
        return out
    if _NC_CACHE is None:
        _NC_CACHE = build_nc()
    nc = _NC_CACHE
    in_maps = host_inputs(idx, token_emb, E, Dx, Dy)
    res = bass_utils.run_bass_kernel_spmd(
        nc, in_maps, core_ids=list(range(2 * B)), **run_kwargs
    )
    out = np.zeros((B, T, D), dtype=np.float32)
    for b in range(B):
        for h in range(2):
            out[b, h * TH:(h + 1) * TH] = res.results[2 * b + h]["out_ft"].T
    if _return_results:
        return out, res
    return out
